# revision 1
# baseline (speedup 1.0000x reference)
"""Causal self-attention with RoPE, tensor-parallel over heads on 8 TRN2 NeuronCores.

Model (from the reference):
    q/k/v = x @ W{q,k,v}.T          x: (1, 2048, 2048), 16 heads x 128 head_dim
    rope(q), rope(k)                half-rotation, 32 nonzero freqs
    causal softmax(q k^T / sqrt(128)) @ v
    out = (y / 3) @ Wo.T

Sharding: 2 heads per core. Each core computes its heads' q/k/v projections,
attention, and a partial c_proj (its 256 columns of the hd contraction);
the host sums the 8 partial outputs (the "all-reduce after c_proj").

Per-core kernel layout choices:
  - Everything transposed so the contraction dim is always on partitions:
    host supplies xT (D, T) plus pre-transposed weight slices.
  - Scores computed transposed (S^T[j, i] blocks) so the P @ V matmul needs
    no transposes: OT[d, i] = sum_j V[j, d]^T P^T[j, i] is produced directly
    in the layout c_proj wants.
  - Softmax without max-subtraction (scores are provably tiny: |s| < ~2),
    denominator via DVE accumulation + one all-ones matmul (broadcast sum).
  - RoPE in transposed layout via a 64-partition roll matmul + 3 DVE ops.
  - All matmuls in float32r (full PE rate at moving dim >= 256).
"""

import numpy as np

T = 2048
D = 2048
H = 16
DH = 128
N_CORES = 8
H_LOC = H // N_CORES          # heads per core = 2
HD_LOC = H_LOC * DH           # local head dims = 256
TCH = 512                     # query-chunk width
N_CH = T // TCH               # 4 chunks
KO = D // 128                 # 16 contraction subtiles
XP = 2                        # xT streamed in pieces of 2 k-subtiles
SCALE = (DH ** 0.5) / DH      # 1/sqrt(128)

_CACHE = {}


def build_program():
    """Build (once) the single-core Bass program shared by all 8 cores."""
    if "nc" in _CACHE:
        return _CACHE["nc"]

    from contextlib import ExitStack

    import concourse.bacc as bacc
    import concourse.mybir as mybir
    import concourse.tile as tile

    f32 = mybir.dt.float32
    f32r = mybir.dt.float32r
    bf16 = mybir.dt.bfloat16
    EXP = mybir.ActivationFunctionType.Exp

    nc = bacc.Bacc("TRN2", target_bir_lowering=False)

    xT_d = nc.dram_tensor("xT", (D, T), f32r, kind="ExternalInput")
    wq_d = nc.dram_tensor("wqT", (D, HD_LOC), f32r, kind="ExternalInput")
    wk_d = nc.dram_tensor("wkT", (D, HD_LOC), f32r, kind="ExternalInput")
    wv_d = nc.dram_tensor("wvT", (D, HD_LOC), f32r, kind="ExternalInput")
    wo_d = nc.dram_tensor("woT", (HD_LOC, D), f32r, kind="ExternalInput")
    ct_d = nc.dram_tensor("ctab", (128, T), f32, kind="ExternalInput")
    st_d = nc.dram_tensor("stab", (128, T), f32, kind="ExternalInput")
    roll_d = nc.dram_tensor("roll", (128, 128), f32r, kind="ExternalInput")
    ones_d = nc.dram_tensor("ones", (128, 128), f32r, kind="ExternalInput")
    tri_d = nc.dram_tensor("tri", (128, 128), f32r, kind="ExternalInput")
    out_d = nc.dram_tensor("outp", (T, D), f32, kind="ExternalOutput")

    xT_r = xT_d[:].rearrange("(ko p) t -> p ko t", p=128)
    wq_r = wq_d[:].rearrange("(ko p) m -> p ko m", p=128)
    wk_r = wk_d[:].rearrange("(ko p) m -> p ko m", p=128)
    wv_r = wv_d[:].rearrange("(ko p) m -> p ko m", p=128)
    wo_r = wo_d[:].rearrange("(h p) d -> p h d", p=128)

    with tile.TileContext(nc) as tc, ExitStack() as ctx:
        persist = ctx.enter_context(tc.tile_pool(name="persist", bufs=1))
        qpool = ctx.enter_context(tc.tile_pool(name="qpool", bufs=2))
        ypool = ctx.enter_context(tc.tile_pool(name="ypool", bufs=2))
        xpool = ctx.enter_context(tc.tile_pool(name="xpool", bufs=10))
        ptpool = ctx.enter_context(tc.tile_pool(name="ptpool", bufs=3))
        rtmp = ctx.enter_context(tc.tile_pool(name="rtmp", bufs=1))
        spool = ctx.enter_context(tc.tile_pool(name="spool", bufs=2))
        opool = ctx.enter_context(tc.tile_pool(name="opool", bufs=6))
        psum_p = ctx.enter_context(tc.tile_pool(name="psum_p", bufs=2, space="PSUM"))
        psum_mix = ctx.enter_context(tc.tile_pool(name="psum_mix", bufs=2, space="PSUM"))
        psum_ot = ctx.enter_context(tc.tile_pool(name="psum_ot", bufs=2, space="PSUM"))

        def ps_tile(pool=None):
            return (pool or psum_p).tile([128, TCH], f32, tag="ps", name="ps")

        def mix_tile():
            return psum_mix.tile([128, H_LOC, TCH], f32, tag="mix", name="mix")

        # --- resident tensors ---
        w_q = persist.tile([128, KO, HD_LOC], f32r, tag="w_q")
        w_k = persist.tile([128, KO, HD_LOC], f32r, tag="w_k")
        w_v = persist.tile([128, KO, HD_LOC], f32r, tag="w_v")
        w_o = persist.tile([128, H_LOC, D], f32r, tag="w_o")
        kt = persist.tile([128, H_LOC, T], f32r, tag="kt")
        vt = persist.tile([128, KO, HD_LOC], f32r, tag="vt")
        ctab = persist.tile([128, T], f32, tag="ctab")
        stab = persist.tile([128, T], f32, tag="stab")
        roll = persist.tile([128, 128], f32r, tag="roll")
        ones = persist.tile([128, 128], f32r, tag="ones")
        tri = persist.tile([128, 128], f32r, tag="tri")

        def issue_x(c):
            """Queue the xT piece DMAs for chunk c (weights too on chunk 0)."""
            cs = c * TCH
            pieces = []
            for kp in range(KO // XP):
                ksl = slice(kp * XP, (kp + 1) * XP)
                xc = xpool.tile([128, XP, TCH], f32r, tag="xc", name="xc")
                nc.sync.dma_start(xc[:], xT_r[:, ksl, cs:cs + TCH])
                pieces.append(xc)
                if c == 0:
                    nc.sync.dma_start(w_q[:, ksl, :], wq_r[:, ksl, :])
                    nc.sync.dma_start(w_k[:, ksl, :], wk_r[:, ksl, :])
                    nc.sync.dma_start(w_v[:, ksl, :], wv_r[:, ksl, :])
            if c == 0:
                nc.sync.dma_start(ctab[:], ct_d[:])
                nc.sync.dma_start(stab[:], st_d[:])
                nc.sync.dma_start(roll[:], roll_d[:])
                nc.sync.dma_start(ones[:], ones_d[:])
                nc.sync.dma_start(tri[:], tri_d[:])
            return pieces

        def proj_chunk(c, pieces, only=None, qc=None):
            """q/k/v projections + RoPE for t-chunk c.

            only="q": just the q projection + its rope (enables starting the
            chunk's early attention j-tiles before k/v exist).
            only="kv": the rest. None: everything."""
            cs = c * TCH
            if only != "kv":
                qc = qpool.tile([128, H_LOC, TCH], f32r, tag="qc", name="qc")
            wd = {"q": ((w_q, qc),), "kv": ((w_k, kt),)}.get(only,
                                                            ((w_q, qc), (w_k, kt)))
            for w_sb, dst in wd:
                for h in range(H_LOC):
                    dsl = dst[:, h, :] if dst is qc else dst[:, h, cs:cs + TCH]
                    # k-groups accumulate in the attention ot pool (idle during
                    # projections) so q/k/roll don't serialize through psum_p;
                    # in split mode that pool is live -- fall back to psum_p
                    ps = ps_tile(psum_ot if (dst is kt and only is None) else None)
                    for ko in range(KO):
                        nc.tensor.matmul(
                            ps,
                            lhsT=w_sb[:, ko, h * 128:(h + 1) * 128],
                            rhs=pieces[ko // XP][:, ko % XP, :],
                            start=(ko == 0),
                            stop=(ko == KO - 1),
                        )
                    nc.scalar.copy(out=dsl, in_=ps)

            if only == "q":
                rope_srcs, do_v = (qc,), False
            elif only == "kv":
                rope_srcs, do_v = (kt,), True
            else:
                rope_srcs, do_v = (qc, kt), True
            # RoPE: y = x*C + roll64(x)*S' (only via PE roll + 3 DVE ops)
            for srct in rope_srcs:
                for h in range(H_LOC):
                    sl = srct[:, h, :] if srct is qc else srct[:, h, cs:cs + TCH]
                    rolled = ps_tile()
                    nc.tensor.matmul(rolled, lhsT=roll, rhs=sl,
                                     start=True, stop=True)
                    a = rtmp.tile([128, TCH], f32, tag="ra", name="ra")
                    b = rtmp.tile([128, TCH], f32, tag="rb", name="rb")
                    nc.vector.tensor_mul(out=a, in0=sl, in1=ctab[:, cs:cs + TCH])
                    nc.vector.tensor_mul(out=b, in0=rolled, in1=stab[:, cs:cs + TCH])
                    nc.vector.tensor_add(out=sl, in0=a, in1=b)
            if not do_v:
                return qc
            # split mode overlaps attention (which owns mix/ot): v uses psum_p
            vmix = mix_tile() if only is None else None
            for tt in range(TCH // 128):
                gt = c * (TCH // 128) + tt
                if vmix is not None:
                    ps = vmix[:, tt // 2,
                              (tt % 2) * HD_LOC:(tt % 2 + 1) * HD_LOC]
                else:
                    ps = ps_tile()
                for ko in range(KO):
                    nc.tensor.matmul(
                        ps[:, :HD_LOC],
                        lhsT=pieces[ko // XP][:, ko % XP, tt * 128:(tt + 1) * 128],
                        rhs=w_v[:, ko, :],
                        start=(ko == 0),
                        stop=(ko == KO - 1),
                    )
                nc.scalar.copy(out=vt[:, gt, :], in_=ps[:, :HD_LOC])

            return qc

        def attn_span(q0, W, qc, off, yc, jt_lo=0, jt_hi=None,
                      state=None):
            """Causal attention for queries [q0, q0+W), heads interleaved.

            q0 must be 128-aligned; W in {256, 512}. qc holds the chunk's
            roped queries; off is q0's offset within qc/yc."""
            d0 = q0 // 128          # first diagonal j-tile
            n_jt = d0 + W // 128
            if state is None:
                ots = [ps_tile(psum_ot) for _ in range(H_LOC)]
                vecsums = [[spool.tile([128, TCH], f32r, tag=f"vecsum{par}",
                                       name="vecsum")
                            for par in range(2)] for _ in range(H_LOC)]
            else:
                ots, vecsums = state
            if jt_hi is None:
                jt_hi = n_jt
            for jt in range(jt_lo, jt_hi):
                pair = mix_tile()
                m = jt - d0
                # diagonal block: cols < 128m fully masked -- never written,
                # never read (partial-width ops)
                lo = 128 * m if m > 0 else 0
                # score matmul skips dead columns too, but only while the
                # moving dim stays >= 256 (full fp32r rate)
                slo = lo if W - lo >= 256 else 0
                for h in range(H_LOC):
                    nc.tensor.matmul(
                        pair[:, h, slo:W],
                        lhsT=kt[:, h, jt * 128:(jt + 1) * 128],
                        rhs=qc[:, h, off + slo:off + W],
                        start=True,
                        stop=True,
                    )
                pt = ptpool.tile([128, H_LOC, TCH], f32r, tag="pt", name="pt")
                # both heads in ONE activation call (strided AP when lo > 0)
                nc.scalar.activation(out=pt[:, :, lo:W], in_=pair[:, :, lo:W],
                                     func=EXP, scale=SCALE)
                for h in range(H_LOC):
                    if m >= 0:
                        nc.vector.tensor_mul(
                            out=pt[:, h, 128 * m:128 * (m + 1)],
                            in0=pt[:, h, 128 * m:128 * (m + 1)],
                            in1=tri[:],
                        )
                    # spans starting at q0=0: jt==1 is diagonal (cols < 128
                    # unwritten), so a full-width init copy would ingest
                    # garbage -- single DVE accumulator there. Other spans
                    # split across DVE (even jt) and GPSIMD (odd jt).
                    par = jt % 2 if d0 >= 2 else 0
                    vs = vecsums[h][par]
                    eng = nc.vector if par == 0 else nc.gpsimd
                    if jt < (2 if d0 >= 2 else 1):
                        eng.tensor_copy(out=vs[:, :W], in_=pt[:, h, :W])
                    else:
                        eng.tensor_add(out=vs[:, lo:W], in0=vs[:, lo:W],
                                       in1=pt[:, h, lo:W])
                    nc.tensor.matmul(
                        ots[h][:, lo:W],
                        lhsT=vt[:, jt, h * 128:(h + 1) * 128],
                        rhs=pt[:, h, lo:W],
                        start=(jt == 0),
                        stop=(jt == n_jt - 1),
                        skip_group_check=(lo > 0),
                    )
            if jt_hi < n_jt:
                return (ots, vecsums)
            for h in range(H_LOC):
                # denominator: all-ones matmul -> column sums on all partitions
                den = mix_tile()[:, 0, :W]
                if d0 >= 2:
                    nc.tensor.matmul(den, lhsT=ones, rhs=vecsums[h][0][:, :W],
                                     start=True, stop=False)
                    nc.tensor.matmul(den, lhsT=ones, rhs=vecsums[h][1][:, :W],
                                     start=False, stop=True)
                else:
                    nc.tensor.matmul(den, lhsT=ones, rhs=vecsums[h][0][:, :W],
                                     start=True, stop=True)
                recipb = rtmp.tile([128, TCH], f32, tag="recipb", name="recipb")
                nc.vector.reciprocal(out=recipb[:, :W], in_=den)
                nc.vector.tensor_mul(out=yc[:, h, off:off + W],
                                     in0=ots[h][:, :W], in1=recipb[:, :W])

        def cproj_span(q0, W, yc, off, copy_eng=None):
            """Partial c_proj (this core's hd columns) for rows [q0, q0+W)."""
            if q0 == 0:
                nc.sync.dma_start(w_o[:], wo_r)
            for tt in range(W // 128):
                gt = q0 // 128 + tt
                for nck in range(D // 512):
                    ps = mix_tile()[:, 0, :]
                    for h in range(H_LOC):
                        nc.tensor.matmul(
                            ps,
                            lhsT=yc[:, h, off + tt * 128:off + (tt + 1) * 128],
                            rhs=w_o[:, h, nck * 512:(nck + 1) * 512],
                            start=(h == 0),
                            stop=(h == H_LOC - 1),
                        )
                    ob = opool.tile([128, 512], f32, tag="ob", name="ob")
                    if copy_eng is None:
                        nc.scalar.copy(out=ob[:], in_=ps)
                    else:
                        copy_eng.tensor_copy(out=ob[:], in_=ps)
                    nc.sync.dma_start(
                        out_d[gt * 128:(gt + 1) * 128,
                              nck * 512:(nck + 1) * 512],
                        ob[:],
                    )

        # Emission order: projections stream in chunk order; each attention
        # chunk is emitted as soon as its projections exist, EXCEPT chunk 0
        # (the smallest, 4 j-tiles) which is saved for the tail so the
        # ACT-bound final attention stretch is as short as possible.
        pieces = issue_x(0)
        for c in range(N_CH - 1):
            qc = proj_chunk(c, pieces)
            pieces = issue_x(c + 1)
            yc = ypool.tile([128, H_LOC, TCH], f32r, tag="yc", name="yc")
            attn_span(c * TCH, TCH, qc, 0, yc)
            cproj_span(c * TCH, TCH, yc, 0)
        # last chunk: q projection + rope first, then its non-diagonal
        # attention (kt/vt chunks 0..2) overlaps the k/v projections
        c = N_CH - 1
        qc = proj_chunk(c, pieces, only="q")
        yc = ypool.tile([128, H_LOC, TCH], f32r, tag="yc", name="yc")
        st = attn_span(c * TCH, TCH, qc, 0, yc, jt_hi=4 * c)
        proj_chunk(c, pieces, only="kv", qc=qc)
        attn_span(c * TCH, TCH, qc, 0, yc, jt_lo=4 * c, state=st)
        cproj_span(c * TCH, TCH, yc, 0)

    nc.compile()
    _CACHE["nc"] = nc
    return nc


def host_inputs(x, Wq, Wk, Wv, Wo):
    """Per-core input dicts (host-side shard + transpose + table prep)."""
    x2 = np.ascontiguousarray(x.reshape(T, D).T).astype(np.float32)  # (D, T)

    half = DH // 2  # 64
    af = (1.0 / 1024.0) ** np.linspace(0.0, 1.0, DH // 4, dtype=np.float32)
    af = np.concatenate([af, np.zeros(DH // 4, np.float32)])         # (64,)
    theta = np.arange(T, dtype=np.float32)[:, None] * af[None, :]    # (T, 64)
    cos = np.cos(theta).T.astype(np.float32)                         # (64, T)
    sin = np.sin(theta).T.astype(np.float32)
    ctab = np.concatenate([cos, cos], axis=0)                        # (128, T)
    stab = np.concatenate([sin, -sin], axis=0)

    roll = np.zeros((128, 128), np.float32)
    for p in range(128):
        roll[p, (p + half) % 128] = 1.0
    ones = np.ones((128, 128), np.float32)
    tri = np.triu(np.ones((128, 128), np.float32))  # tri[j, i] = i >= j

    shared = {
        "xT": x2, "ctab": ctab, "stab": stab,
        "roll": roll, "ones": ones, "tri": tri,
    }
    in_maps = []
    for c in range(N_CORES):
        sl = slice(c * HD_LOC, (c + 1) * HD_LOC)
        in_maps.append({
            **shared,
            "wqT": np.ascontiguousarray(Wq[sl, :].T),
            "wkT": np.ascontiguousarray(Wk[sl, :].T),
            "wvT": np.ascontiguousarray(Wv[sl, :].T),
            "woT": np.ascontiguousarray((Wo[:, sl] / 3.0).T),
        })
    return in_maps


def _get_runner():
    """Build the program + a persistent jitted SPMD executable (once)."""
    if "runner" in _CACHE:
        return _CACHE["runner"]

    import jax
    import concourse.mybir as mybir
    from concourse.bass2jax import (
        _bass_exec_p,
        install_neuronx_cc_hook,
        partition_id_tensor,
    )
    from jax.experimental.shard_map import shard_map
    from jax.sharding import Mesh, PartitionSpec

    nc = build_program()
    install_neuronx_cc_hook()
    assert nc.dbg_addr is None
    pid_name = nc.partition_id_tensor.name if nc.partition_id_tensor else None

    in_names, out_names, out_avals, zero_outs = [], [], [], []
    for alloc in nc.m.functions[0].allocations:
        if not isinstance(alloc, mybir.MemoryLocationSet):
            continue
        name = alloc.memorylocations[0].name
        if alloc.kind == "ExternalInput":
            if name != pid_name:
                in_names.append(name)
        elif alloc.kind == "ExternalOutput":
            out_names.append(name)
            shape = tuple(alloc.tensor_shape)
            dtype = mybir.dt.np(alloc.dtype)
            out_avals.append(jax.core.ShapedArray(shape, dtype))
            zero_outs.append(np.zeros(shape, dtype))
    n_params = len(in_names)
    all_names = list(in_names) + list(out_names)
    if pid_name is not None:
        all_names.append(pid_name)
    donate = tuple(range(n_params, n_params + len(out_names)))

    def _body(*args):
        operands = list(args)
        if pid_name is not None:
            operands.append(partition_id_tensor())
        outs = _bass_exec_p.bind(
            *operands,
            out_avals=tuple(out_avals),
            in_names=tuple(all_names),
            out_names=tuple(out_names),
            lowering_input_output_aliases=(),
            sim_require_finite=True,
            sim_require_nnan=True,
            nc=nc,
        )
        return tuple(outs)

    devices = jax.devices()[:N_CORES]
    mesh = Mesh(np.asarray(devices), ("core",))
    in_specs = (PartitionSpec("core"),) * (n_params + len(out_names))
    out_specs = (PartitionSpec("core"),) * len(out_names)
    fn = jax.jit(
        shard_map(_body, mesh=mesh, in_specs=in_specs, out_specs=out_specs,
                  check_rep=False),
        donate_argnums=donate,
        keep_unused=True,
    )
    runner = (fn, in_names, out_names, out_avals, zero_outs)
    _CACHE["runner"] = runner
    return runner


def run_spmd(in_maps):
    """Execute the SPMD program; returns per-core output dicts."""
    fn, in_names, out_names, out_avals, zero_outs = _get_runner()
    concat_in = [
        np.concatenate([np.asarray(in_maps[c][n]) for c in range(N_CORES)], axis=0)
        for n in in_names
    ]
    concat_zeros = [
        np.zeros((N_CORES * z.shape[0], *z.shape[1:]), z.dtype) for z in zero_outs
    ]
    out_arrs = fn(*concat_in, *concat_zeros)
    return [
        {n: np.asarray(out_arrs[i]).reshape(N_CORES, *out_avals[i].shape)[c]
         for i, n in enumerate(out_names)}
        for c in range(N_CORES)
    ]


def kernel(x, Wq, Wk, Wv, Wo):
    in_maps = host_inputs(np.asarray(x), np.asarray(Wq), np.asarray(Wk),
                          np.asarray(Wv), np.asarray(Wo))
    results = run_spmd(in_maps)
    out = results[0]["outp"].astype(np.float64)
    for c in range(1, N_CORES):
        out += results[c]["outp"]
    return out.astype(np.float32).reshape(1, T, D)



# revision 2
# speedup vs baseline: 1.1057x; 1.1057x over previous
"""Causal self-attention with RoPE, tensor-parallel over heads on 8 TRN2 NeuronCores.

Model (from the reference):
    q/k/v = x @ W{q,k,v}.T          x: (1, 2048, 2048), 16 heads x 128 head_dim
    rope(q), rope(k)                half-rotation, 32 nonzero freqs
    causal softmax(q k^T / sqrt(128)) @ v
    out = (y / 3) @ Wo.T

Sharding: 2 heads per core. Each core computes its heads' q/k/v projections,
attention, and a partial c_proj (its 256 columns of the hd contraction);
the host sums the 8 partial outputs (the "all-reduce after c_proj").

Numerics (validated against the reference on the real inputs, final
max-err/absmax ~8e-3 vs the 2e-2 gate):
  - q/k projections: raw fp8e4m3 (x_hi, w*2^8) via DoubleRow matmuls
    (2x128 contraction per instruction at 0.5 cycles/row -> 4x PE rate).
    Score errors are damped because |scores| is small, so attention
    probabilities see only a small absolute perturbation.
  - v projection and c_proj: 3-term compensated fp8 (x_hi@w_hi + x_hi@w_lo
    + x_lo@w_hi), each term a DoubleRow matmul -> 1.33x PE rate with
    ~1e-3 final error. These feed the output linearly, so raw fp8 would
    blow the error budget.
  - scores: fp8 operands (quantized by the rope's final DVE add), plain
    matmuls. Same PE rate as fp32r but no <256-wide penalty, so the causal
    diagonal is trimmed at 128 granularity.
  - P (exp scores) and V: bf16 -> PV matmuls at full rate any width, DVE
    masking/accumulation in 2x mode, half the SBUF.
  - RoPE roll (partition rotation by 64) done by two SBUF->SBUF DMAs
    instead of a PE matmul; rope mul/add on DVE in bf16.
  - Output partials in bf16 (summed across cores on the host in f64).

Layout: everything transposed so the contraction dim is on partitions;
scores computed as S^T so P^T @ V needs no transposes; softmax without
max-subtraction (scores are provably tiny); denominator via bf16 vecsum
accumulation (DVE+GPSIMD) + an all-(1/16) matmul, the 16 folded back in
the reciprocal, giving y*16 which fp8 splits cleanly for c_proj.
"""

import numpy as np

T = 2048
D = 2048
H = 16
DH = 128
N_CORES = 8
H_LOC = H // N_CORES          # heads per core = 2
HD_LOC = H_LOC * DH           # local head dims = 256
TCH = 512                     # query-chunk width
N_CH = T // TCH               # 4 chunks
KO = D // 128                 # 16 contraction subtiles
XP = 2                        # xT streamed in pieces of 2 k-subtiles
KP = KO // XP                 # 8 DoubleRow pairs over the contraction
SCALE = (DH ** 0.5) / DH      # 1/sqrt(128)
WSC = 256.0                   # fp8 weight pre-scale (2^8)
YSC = 16.0                    # y pre-scale folded into the reciprocal

_CACHE = {}


def build_program():
    """Build (once) the single-core Bass program shared by all 8 cores."""
    if "nc" in _CACHE:
        return _CACHE["nc"]

    from contextlib import ExitStack

    import concourse.bacc as bacc
    import concourse.mybir as mybir
    import concourse.tile as tile

    f32 = mybir.dt.float32
    bf16 = mybir.dt.bfloat16
    f8 = mybir.dt.float8e4
    EXP = mybir.ActivationFunctionType.Exp
    COPY = mybir.ActivationFunctionType.Copy
    DR = mybir.MatmulPerfMode.DoubleRow

    nc = bacc.Bacc("TRN2", target_bir_lowering=False)

    x8h_d = nc.dram_tensor("x8h", (D, T), f8, kind="ExternalInput")
    x8l_d = nc.dram_tensor("x8l", (D, T), f8, kind="ExternalInput")
    wq_d = nc.dram_tensor("wq8", (D, HD_LOC), f8, kind="ExternalInput")
    wk_d = nc.dram_tensor("wk8", (D, HD_LOC), f8, kind="ExternalInput")
    wvh_d = nc.dram_tensor("wv8h", (D, HD_LOC), f8, kind="ExternalInput")
    wvl_d = nc.dram_tensor("wv8l", (D, HD_LOC), f8, kind="ExternalInput")
    woh_d = nc.dram_tensor("wo8h", (HD_LOC, D), f8, kind="ExternalInput")
    wol_d = nc.dram_tensor("wo8l", (HD_LOC, D), f8, kind="ExternalInput")
    ct_d = nc.dram_tensor("ctab", (128, T), bf16, kind="ExternalInput")
    st_d = nc.dram_tensor("stab", (128, T), bf16, kind="ExternalInput")
    ones_d = nc.dram_tensor("ones", (128, 128), bf16, kind="ExternalInput")
    tri_d = nc.dram_tensor("tri", (128, 128), bf16, kind="ExternalInput")
    out_d = nc.dram_tensor("outp", (T, D), bf16, kind="ExternalOutput")

    x8h_r = x8h_d[:].rearrange("(ko p) t -> p ko t", p=128)
    x8l_r = x8l_d[:].rearrange("(ko p) t -> p ko t", p=128)
    wq_r = wq_d[:].rearrange("(ko p) m -> p ko m", p=128)
    wk_r = wk_d[:].rearrange("(ko p) m -> p ko m", p=128)
    wvh_r = wvh_d[:].rearrange("(ko p) m -> p ko m", p=128)
    wvl_r = wvl_d[:].rearrange("(ko p) m -> p ko m", p=128)
    woh_r = woh_d[:].rearrange("(h p) d -> p h d", p=128)
    wol_r = wol_d[:].rearrange("(h p) d -> p h d", p=128)

    with tile.TileContext(nc) as tc, ExitStack() as ctx:
        persist = ctx.enter_context(tc.tile_pool(name="persist", bufs=1))
        qpool = ctx.enter_context(tc.tile_pool(name="qpool", bufs=2))
        ypool = ctx.enter_context(tc.tile_pool(name="ypool", bufs=2))
        xpool = ctx.enter_context(tc.tile_pool(name="xpool", bufs=10))
        ptpool = ctx.enter_context(tc.tile_pool(name="ptpool", bufs=3))
        rtmp = ctx.enter_context(tc.tile_pool(name="rtmp", bufs=2))
        spool = ctx.enter_context(tc.tile_pool(name="spool", bufs=2))
        opool = ctx.enter_context(tc.tile_pool(name="opool", bufs=6))
        psum_p = ctx.enter_context(tc.tile_pool(name="psum_p", bufs=2, space="PSUM"))
        psum_mix = ctx.enter_context(tc.tile_pool(name="psum_mix", bufs=2, space="PSUM"))
        psum_ot = ctx.enter_context(tc.tile_pool(name="psum_ot", bufs=2, space="PSUM"))

        def ps_tile(pool=None):
            return (pool or psum_p).tile([128, TCH], f32, tag="ps", name="ps")

        def mix_tile():
            return psum_mix.tile([128, H_LOC, TCH], f32, tag="mix", name="mix")

        # --- resident tensors ---
        w_q = persist.tile([128, KO, HD_LOC], f8, tag="w_q")
        w_k = persist.tile([128, KO, HD_LOC], f8, tag="w_k")
        w_vh = persist.tile([128, KO, HD_LOC], f8, tag="w_vh")
        w_vl = persist.tile([128, KO, HD_LOC], f8, tag="w_vl")
        w_oh = persist.tile([128, H_LOC, D], f8, tag="w_oh")
        w_ol = persist.tile([128, H_LOC, D], f8, tag="w_ol")
        kt8 = persist.tile([128, H_LOC, T], f8, tag="kt8")
        vt = persist.tile([128, KO, HD_LOC], bf16, tag="vt")
        ctab = persist.tile([128, T], bf16, tag="ctab")
        stab = persist.tile([128, T], bf16, tag="stab")
        ones = persist.tile([128, 128], bf16, tag="ones")
        tri = persist.tile([128, 128], bf16, tag="tri")

        def issue_x(c):
            """Queue the x piece DMAs for chunk c (weights too on chunk 0)."""
            cs = c * TCH
            pieces = []
            for kp in range(KP):
                ksl = slice(kp * XP, (kp + 1) * XP)
                xh = xpool.tile([128, XP, TCH], f8, tag="xh", name="xh")
                xl = xpool.tile([128, XP, TCH], f8, tag="xl", name="xl")
                nc.sync.dma_start(xh[:], x8h_r[:, ksl, cs:cs + TCH])
                nc.sync.dma_start(xl[:], x8l_r[:, ksl, cs:cs + TCH])
                pieces.append((xh, xl))
                if c == 0:
                    nc.sync.dma_start(w_q[:, ksl, :], wq_r[:, ksl, :])
                    nc.sync.dma_start(w_k[:, ksl, :], wk_r[:, ksl, :])
                    nc.sync.dma_start(w_vh[:, ksl, :], wvh_r[:, ksl, :])
                    nc.sync.dma_start(w_vl[:, ksl, :], wvl_r[:, ksl, :])
            if c == 0:
                nc.sync.dma_start(ctab[:], ct_d[:])
                nc.sync.dma_start(stab[:], st_d[:])
                nc.sync.dma_start(ones[:], ones_d[:])
                nc.sync.dma_start(tri[:], tri_d[:])
            return pieces

        def proj_chunk(c, pieces, only=None, qc=None):
            """q/k/v projections + RoPE for t-chunk c.

            only="q": just the q projection + its rope (enables starting the
            chunk's early attention j-tiles before k/v exist).
            only="kv": the rest. None: everything."""
            cs = c * TCH
            if only != "kv":
                qc = qpool.tile([128, H_LOC, TCH], f8, tag="qc", name="qc")
            wd = {"q": ((w_q, qc),), "kv": ((w_k, kt8),)}.get(only,
                                                             ((w_q, qc), (w_k, kt8)))
            for w_sb, dst in wd:
                # pre-rope staging tile (bf16) for this src
                pre = rtmp.tile([128, H_LOC, TCH], bf16, tag="pre", name="pre")
                for h in range(H_LOC):
                    # k-groups accumulate in the attention ot pool (idle during
                    # projections); in split mode that pool is live -- psum_p
                    ps = ps_tile(psum_ot if (dst is kt8 and only is None) else None)
                    for kp in range(KP):
                        nc.tensor.matmul(
                            ps,
                            lhsT=w_sb[:, kp * XP:(kp + 1) * XP,
                                      h * 128:(h + 1) * 128],
                            rhs=pieces[kp][0][:],
                            start=(kp == 0),
                            stop=(kp == KP - 1),
                            perf_mode=DR,
                        )
                    nc.scalar.activation(out=pre[:, h, :], in_=ps, func=COPY,
                                         scale=1.0 / WSC)
                # RoPE: y = pre*C + roll64(pre)*S', roll via 2 SBUF->SBUF DMAs
                rolled = rtmp.tile([128, H_LOC, TCH], bf16, tag="rolled",
                                   name="rolled")
                for h in range(H_LOC):
                    nc.sync.dma_start(rolled[0:64, h, :], pre[64:128, h, :])
                    nc.sync.dma_start(rolled[64:128, h, :], pre[0:64, h, :])
                a = rtmp.tile([128, H_LOC, TCH], bf16, tag="ra", name="ra")
                b = rtmp.tile([128, H_LOC, TCH], bf16, tag="rb", name="rb")
                for h in range(H_LOC):
                    nc.vector.tensor_mul(out=a[:, h, :], in0=pre[:, h, :],
                                         in1=ctab[:, cs:cs + TCH])
                    nc.vector.tensor_mul(out=b[:, h, :], in0=rolled[:, h, :],
                                         in1=stab[:, cs:cs + TCH])
                dsl = qc[:, :, :] if dst is qc else kt8[:, :, cs:cs + TCH]
                nc.vector.tensor_add(out=dsl, in0=a[:], in1=b[:])

            if only == "q":
                return qc
            # v projection: 3-term compensated fp8 (hi@hi + hi@lo + lo@hi)
            vmix = mix_tile() if only is None else None
            for tt in range(TCH // 128):
                gt = c * (TCH // 128) + tt
                tsl = slice(tt * 128, (tt + 1) * 128)
                if vmix is not None:
                    ps = vmix[:, tt // 2,
                              (tt % 2) * HD_LOC:(tt % 2 + 1) * HD_LOC]
                else:
                    ps = ps_tile()[:, :HD_LOC]
                n = 3 * KP
                i = 0
                for xi, wv in ((0, w_vh), (0, w_vl), (1, w_vh)):
                    for kp in range(KP):
                        ksl = slice(kp * XP, (kp + 1) * XP)
                        nc.tensor.matmul(
                            ps,
                            lhsT=pieces[kp][xi][:, :, tsl],
                            rhs=wv[:, ksl, :],
                            start=(i == 0),
                            stop=(i == n - 1),
                            perf_mode=DR,
                        )
                        i += 1
                nc.vector.tensor_scalar_mul(vt[:, gt, :], ps, 1.0 / WSC)

            return qc

        def attn_span(q0, W, qc, off, yc, jt_lo=0, jt_hi=None,
                      state=None):
            """Causal attention for queries [q0, q0+W), heads interleaved.

            q0 must be 128-aligned; W in {256, 512}. qc holds the chunk's
            roped queries (fp8); off is q0's offset within qc/yc."""
            d0 = q0 // 128          # first diagonal j-tile
            n_jt = d0 + W // 128
            if state is None:
                ots = [ps_tile(psum_ot) for _ in range(H_LOC)]
                vecsums = [spool.tile([128, H_LOC, TCH], bf16,
                                      tag=f"vecsum{par}", name="vecsum")
                           for par in range(2)]
            else:
                ots, vecsums = state
            if jt_hi is None:
                jt_hi = n_jt
            for jt in range(jt_lo, jt_hi):
                pair = mix_tile()
                m = jt - d0
                # diagonal block: cols < 128m fully masked -- never written,
                # never read (partial-width ops; fp8/bf16 have no narrow-
                # matmul penalty, so trim at full 128 granularity)
                lo = 128 * m if m > 0 else 0
                for h in range(H_LOC):
                    nc.tensor.matmul(
                        pair[:, h, lo:W],
                        lhsT=kt8[:, h, jt * 128:(jt + 1) * 128],
                        rhs=qc[:, h, off + lo:off + W],
                        start=True,
                        stop=True,
                    )
                pt = ptpool.tile([128, H_LOC, TCH], bf16, tag="pt", name="pt")
                # both heads in ONE activation call (strided AP when lo > 0)
                nc.scalar.activation(out=pt[:, :, lo:W], in_=pair[:, :, lo:W],
                                     func=EXP, scale=SCALE)
                for h in range(H_LOC):
                    if m >= 0:
                        nc.vector.tensor_mul(
                            out=pt[:, h, 128 * m:128 * (m + 1)],
                            in0=pt[:, h, 128 * m:128 * (m + 1)],
                            in1=tri[:],
                        )
                # spans starting at q0=0: jt==1 is diagonal (cols < 128
                # unwritten), so a full-width init copy would ingest
                # garbage -- single DVE accumulator there. Other spans
                # split across DVE (even jt) and GPSIMD (odd jt).
                par = jt % 2 if d0 >= 2 else 0
                vs = vecsums[par]
                eng = nc.vector if par == 0 else nc.gpsimd
                if jt < (2 if d0 >= 2 else 1):
                    eng.tensor_copy(out=vs[:, :, :W], in_=pt[:, :, :W])
                else:
                    eng.tensor_add(out=vs[:, :, lo:W], in0=vs[:, :, lo:W],
                                   in1=pt[:, :, lo:W])
                for h in range(H_LOC):
                    nc.tensor.matmul(
                        ots[h][:, lo:W],
                        lhsT=vt[:, jt, h * 128:(h + 1) * 128],
                        rhs=pt[:, h, lo:W],
                        start=(jt == 0),
                        stop=(jt == n_jt - 1),
                        skip_group_check=(lo > 0),
                    )
            if jt_hi < n_jt:
                return (ots, vecsums)
            # denominator: all-(1/16) matmul -> column sums/16 on all
            # partitions; the 16 resurfaces via the reciprocal so yc = 16*y
            den = mix_tile()
            for h in range(H_LOC):
                if d0 >= 2:
                    nc.tensor.matmul(den[:, h, :W], lhsT=ones,
                                     rhs=vecsums[0][:, h, :W],
                                     start=True, stop=False)
                    nc.tensor.matmul(den[:, h, :W], lhsT=ones,
                                     rhs=vecsums[1][:, h, :W],
                                     start=False, stop=True)
                else:
                    nc.tensor.matmul(den[:, h, :W], lhsT=ones,
                                     rhs=vecsums[0][:, h, :W],
                                     start=True, stop=True)
            recipb = rtmp.tile([128, H_LOC, TCH], f32, tag="recipb",
                               name="recipb")
            nc.vector.reciprocal(out=recipb[:, :, :W], in_=den[:, :, :W])
            for h in range(H_LOC):
                nc.vector.tensor_mul(out=yc[:, h, off:off + W],
                                     in0=ots[h][:, :W],
                                     in1=recipb[:, h, :W])

        def cproj_span(q0, W, yc):
            """Partial c_proj (this core's hd columns) for rows [q0, q0+W).

            yc holds 16*y in f32; split into fp8 hi+lo on GPSIMD, then
            3-term DoubleRow matmuls (both heads packed per instruction)."""
            if q0 == 0:
                nc.sync.dma_start(w_oh[:], woh_r)
                nc.sync.dma_start(w_ol[:], wol_r)
            y8h = ypool.tile([128, H_LOC, TCH], f8, tag="y8h", name="y8h")
            y8l = ypool.tile([128, H_LOC, TCH], f8, tag="y8l", name="y8l")
            nc.gpsimd.tensor_copy(out=y8h[:], in_=yc[:])
            nc.gpsimd.tensor_sub(out=y8l[:], in0=yc[:], in1=y8h[:])
            for tt in range(W // 128):
                gt = q0 // 128 + tt
                tsl = slice(tt * 128, (tt + 1) * 128)
                for nck in range(D // 1024):
                    ps = mix_tile()
                    for half in range(2):
                        dsl = slice(nck * 1024 + half * 512,
                                    nck * 1024 + (half + 1) * 512)
                        for yy, ww, st, sp in ((y8h, w_oh, True, False),
                                               (y8h, w_ol, False, False),
                                               (y8l, w_oh, False, True)):
                            nc.tensor.matmul(
                                ps[:, half, :],
                                lhsT=yy[:, :, tsl],
                                rhs=ww[:, :, dsl],
                                start=st,
                                stop=sp,
                                perf_mode=DR,
                            )
                    ob = opool.tile([128, 2, 512], bf16, tag="ob", name="ob")
                    nc.scalar.activation(out=ob[:], in_=ps[:], func=COPY,
                                         scale=1.0 / (WSC * YSC))
                    nc.sync.dma_start(
                        out_d[gt * 128:(gt + 1) * 128,
                              nck * 1024:(nck + 1) * 1024],
                        ob[:].rearrange("p a b -> p (a b)"),
                    )

        # Emission order: projections stream in chunk order; each attention
        # chunk is emitted as soon as its projections exist, EXCEPT chunk 0
        # (the smallest, 4 j-tiles) which is saved for the tail so the
        # ACT-bound final attention stretch is as short as possible.
        pieces = issue_x(0)
        for c in range(N_CH - 1):
            qc = proj_chunk(c, pieces)
            pieces = issue_x(c + 1)
            yc = ypool.tile([128, H_LOC, TCH], f32, tag="yc", name="yc")
            attn_span(c * TCH, TCH, qc, 0, yc)
            cproj_span(c * TCH, TCH, yc)
        # last chunk: q projection + rope first, then its non-diagonal
        # attention (kt/vt chunks 0..2) overlaps the k/v projections
        c = N_CH - 1
        qc = proj_chunk(c, pieces, only="q")
        yc = ypool.tile([128, H_LOC, TCH], f32, tag="yc", name="yc")
        st = attn_span(c * TCH, TCH, qc, 0, yc, jt_hi=4 * c)
        proj_chunk(c, pieces, only="kv", qc=qc)
        attn_span(c * TCH, TCH, qc, 0, yc, jt_lo=4 * c, state=st)
        cproj_span(c * TCH, TCH, yc)

    nc.compile()
    _CACHE["nc"] = nc
    return nc


def host_inputs(x, Wq, Wk, Wv, Wo):
    """Per-core input dicts (host-side shard + transpose + fp8 split)."""
    import ml_dtypes

    F8 = ml_dtypes.float8_e4m3
    BF = ml_dtypes.bfloat16

    def f8_of(a):
        return np.asarray(a, np.float32).astype(F8)

    def f8_split(a):
        hi = f8_of(a)
        lo = (np.asarray(a, np.float32) - hi.astype(np.float32)).astype(F8)
        return hi, lo

    x2 = np.ascontiguousarray(x.reshape(T, D).T).astype(np.float32)  # (D, T)
    x8h, x8l = f8_split(x2)

    af = (1.0 / 1024.0) ** np.linspace(0.0, 1.0, DH // 4, dtype=np.float32)
    af = np.concatenate([af, np.zeros(DH // 4, np.float32)])         # (64,)
    theta = np.arange(T, dtype=np.float32)[:, None] * af[None, :]    # (T, 64)
    cos = np.cos(theta).T.astype(np.float32)                         # (64, T)
    sin = np.sin(theta).T.astype(np.float32)
    ctab = np.concatenate([cos, cos], axis=0).astype(BF)             # (128, T)
    stab = np.concatenate([sin, -sin], axis=0).astype(BF)

    ones = np.full((128, 128), 1.0 / YSC, BF)
    tri = np.triu(np.ones((128, 128), np.float32)).astype(BF)  # tri[j,i]=i>=j

    shared = {
        "x8h": x8h, "x8l": x8l, "ctab": ctab, "stab": stab,
        "ones": ones, "tri": tri,
    }
    in_maps = []
    for c in range(N_CORES):
        sl = slice(c * HD_LOC, (c + 1) * HD_LOC)
        wv8h, wv8l = f8_split(Wv[sl, :].T * WSC)
        wo8h, wo8l = f8_split((Wo[:, sl] / 3.0).T * WSC)
        in_maps.append({
            **shared,
            "wq8": f8_of(Wq[sl, :].T * WSC),
            "wk8": f8_of(Wk[sl, :].T * WSC),
            "wv8h": wv8h, "wv8l": wv8l,
            "wo8h": wo8h, "wo8l": wo8l,
        })
    return in_maps


def _get_runner():
    """Build the program + a persistent jitted SPMD executable (once)."""
    if "runner" in _CACHE:
        return _CACHE["runner"]

    import jax
    import concourse.mybir as mybir
    from concourse.bass2jax import (
        _bass_exec_p,
        install_neuronx_cc_hook,
        partition_id_tensor,
    )
    from jax.experimental.shard_map import shard_map
    from jax.sharding import Mesh, PartitionSpec

    nc = build_program()
    install_neuronx_cc_hook()
    assert nc.dbg_addr is None
    pid_name = nc.partition_id_tensor.name if nc.partition_id_tensor else None

    in_names, out_names, out_avals, zero_outs = [], [], [], []
    for alloc in nc.m.functions[0].allocations:
        if not isinstance(alloc, mybir.MemoryLocationSet):
            continue
        name = alloc.memorylocations[0].name
        if alloc.kind == "ExternalInput":
            if name != pid_name:
                in_names.append(name)
        elif alloc.kind == "ExternalOutput":
            out_names.append(name)
            shape = tuple(alloc.tensor_shape)
            dtype = mybir.dt.np(alloc.dtype)
            out_avals.append(jax.core.ShapedArray(shape, dtype))
            zero_outs.append(np.zeros(shape, dtype))
    n_params = len(in_names)
    all_names = list(in_names) + list(out_names)
    if pid_name is not None:
        all_names.append(pid_name)
    donate = tuple(range(n_params, n_params + len(out_names)))

    def _body(*args):
        operands = list(args)
        if pid_name is not None:
            operands.append(partition_id_tensor())
        outs = _bass_exec_p.bind(
            *operands,
            out_avals=tuple(out_avals),
            in_names=tuple(all_names),
            out_names=tuple(out_names),
            lowering_input_output_aliases=(),
            sim_require_finite=True,
            sim_require_nnan=True,
            nc=nc,
        )
        return tuple(outs)

    devices = jax.devices()[:N_CORES]
    mesh = Mesh(np.asarray(devices), ("core",))
    in_specs = (PartitionSpec("core"),) * (n_params + len(out_names))
    out_specs = (PartitionSpec("core"),) * len(out_names)
    fn = jax.jit(
        shard_map(_body, mesh=mesh, in_specs=in_specs, out_specs=out_specs,
                  check_rep=False),
        donate_argnums=donate,
        keep_unused=True,
    )
    runner = (fn, in_names, out_names, out_avals, zero_outs)
    _CACHE["runner"] = runner
    return runner


def run_spmd(in_maps):
    """Execute the SPMD program; returns per-core output dicts."""
    fn, in_names, out_names, out_avals, zero_outs = _get_runner()
    concat_in = [
        np.concatenate([np.asarray(in_maps[c][n]) for c in range(N_CORES)], axis=0)
        for n in in_names
    ]
    concat_zeros = [
        np.zeros((N_CORES * z.shape[0], *z.shape[1:]), z.dtype) for z in zero_outs
    ]
    out_arrs = fn(*concat_in, *concat_zeros)
    return [
        {n: np.asarray(out_arrs[i]).reshape(N_CORES, *out_avals[i].shape)[c]
         for i, n in enumerate(out_names)}
        for c in range(N_CORES)
    ]


def kernel(x, Wq, Wk, Wv, Wo):
    in_maps = host_inputs(np.asarray(x), np.asarray(Wq), np.asarray(Wk),
                          np.asarray(Wv), np.asarray(Wo))
    results = run_spmd(in_maps)
    out = results[0]["outp"].astype(np.float64)
    for c in range(1, N_CORES):
        out += results[c]["outp"].astype(np.float64)
    return out.astype(np.float32).reshape(1, T, D)


# revision 12
# speedup vs baseline: 1.2709x; 1.1494x over previous
"""Causal self-attention with RoPE, tensor-parallel over heads on 8 TRN2 NeuronCores.

Model (from the reference):
    q/k/v = x @ W{q,k,v}.T          x: (1, 2048, 2048), 16 heads x 128 head_dim
    rope(q), rope(k)                half-rotation, 32 nonzero freqs
    causal softmax(q k^T / sqrt(128)) @ v
    out = (y / 3) @ Wo.T

Sharding: 2 heads per core. Each core computes its heads' q/k/v projections,
attention, and a partial c_proj (its 256 columns of the hd contraction);
the host sums the 8 partial outputs (the "all-reduce after c_proj").

Numerics (validated against the reference on the real inputs, final
max-err/absmax ~8e-3 vs the 2e-2 gate):
  - q/k projections: raw fp8e4m3 (x_hi, w*2^8) via DoubleRow matmuls
    (2x128 contraction per instruction at 0.5 cycles/row -> 4x PE rate).
    Score errors are damped because |scores| is small, so attention
    probabilities see only a small absolute perturbation.
  - v projection and c_proj: 3-term compensated fp8 (x_hi@w_hi + x_hi@w_lo
    + x_lo@w_hi), each term a DoubleRow matmul -> 1.33x PE rate with
    ~1e-3 final error. These feed the output linearly, so raw fp8 would
    blow the error budget.
  - scores: fp8 operands (quantized by the rope's final DVE add), plain
    matmuls. Same PE rate as fp32r but no <256-wide penalty, so the causal
    diagonal is trimmed at 128 granularity.
  - P (exp scores) and V: bf16 -> PV matmuls at full rate any width, DVE
    masking/accumulation in 2x mode, half the SBUF.
  - RoPE roll (partition rotation by 64) done by two SBUF->SBUF DMAs
    instead of a PE matmul; rope mul/add on DVE in bf16.
  - Output partials in bf16 (summed across cores on the host in f64).

Layout: everything transposed so the contraction dim is on partitions;
scores computed as S^T so P^T @ V needs no transposes; softmax without
max-subtraction (scores are provably tiny); denominator via bf16 vecsum
accumulation (DVE+GPSIMD) + an all-(1/16) matmul, the 16 folded back in
the reciprocal, giving y*16 which fp8 splits cleanly for c_proj.
"""

import numpy as np

T = 2048
D = 2048
H = 16
DH = 128
N_CORES = 8
H_LOC = H // N_CORES          # heads per core = 2
HD_LOC = H_LOC * DH           # local head dims = 256
TCH = 512                     # query-chunk width
N_CH = T // TCH               # 4 chunks
KO = D // 128                 # 16 contraction subtiles
XP = 4                        # xT streamed in pieces of 4 k-subtiles
KP = KO // 2                  # 8 DoubleRow pairs over the contraction
SCALE = (DH ** 0.5) / DH      # 1/sqrt(128)
WSC = 256.0                   # fp8 weight pre-scale (2^8)
YSC = 16.0                    # y pre-scale folded into the reciprocal

_CACHE = {}


def build_program():
    """Build (once) the single-core Bass program shared by all 8 cores."""
    if "nc" in _CACHE:
        return _CACHE["nc"]

    from contextlib import ExitStack

    import concourse.bacc as bacc
    import concourse.mybir as mybir
    import concourse.tile as tile

    f32 = mybir.dt.float32
    bf16 = mybir.dt.bfloat16
    f8 = mybir.dt.float8e4
    EXP = mybir.ActivationFunctionType.Exp
    COPY = mybir.ActivationFunctionType.Copy
    DR = mybir.MatmulPerfMode.DoubleRow

    nc = bacc.Bacc("TRN2", target_bir_lowering=False)

    # weights arrive pre-rearranged to partition-major layouts so each loads
    # with ONE full-rate DMA (4KB+ contiguous per partition)
    x8h_d = nc.dram_tensor("x8h", (D, T), f8, kind="ExternalInput")
    x8l_d = nc.dram_tensor("x8l", (D, T), f8, kind="ExternalInput")
    wq_d = nc.dram_tensor("wq8", (128, KO * HD_LOC), f8, kind="ExternalInput")
    wk_d = nc.dram_tensor("wk8", (128, KO * HD_LOC), f8, kind="ExternalInput")
    wvh_d = nc.dram_tensor("wv8h", (128, KO * HD_LOC), f8, kind="ExternalInput")
    wvl_d = nc.dram_tensor("wv8l", (128, KO * HD_LOC), f8, kind="ExternalInput")
    woh_d = nc.dram_tensor("wo8h", (128, H_LOC * D), f8, kind="ExternalInput")
    wol_d = nc.dram_tensor("wo8l", (128, H_LOC * D), f8, kind="ExternalInput")
    ct_d = nc.dram_tensor("ctab", (128, T), bf16, kind="ExternalInput")
    st_d = nc.dram_tensor("stab", (128, T), bf16, kind="ExternalInput")
    ones_d = nc.dram_tensor("ones", (128, 128), bf16, kind="ExternalInput")
    tri_d = nc.dram_tensor("tri", (128, 128), bf16, kind="ExternalInput")
    out_d = nc.dram_tensor("outp", (T, D), bf16, kind="ExternalOutput")

    x8h_r = x8h_d[:].rearrange("(ko p) t -> p ko t", p=128)
    x8l_r = x8l_d[:].rearrange("(ko p) t -> p ko t", p=128)
    wq_r = wq_d[:].rearrange("p (ko m) -> p ko m", ko=KO)
    wk_r = wk_d[:].rearrange("p (ko m) -> p ko m", ko=KO)
    wvh_r = wvh_d[:].rearrange("p (ko m) -> p ko m", ko=KO)
    wvl_r = wvl_d[:].rearrange("p (ko m) -> p ko m", ko=KO)
    woh_r = woh_d[:].rearrange("p (h d) -> p h d", h=H_LOC)
    wol_r = wol_d[:].rearrange("p (h d) -> p h d", h=H_LOC)

    with tile.TileContext(nc) as tc, ExitStack() as ctx:
        persist = ctx.enter_context(tc.tile_pool(name="persist", bufs=1))
        qpool = ctx.enter_context(tc.tile_pool(name="qpool", bufs=2))
        ypool = ctx.enter_context(tc.tile_pool(name="ypool", bufs=2))
        xpool = ctx.enter_context(tc.tile_pool(name="xpool", bufs=10))
        ptpool = ctx.enter_context(tc.tile_pool(name="ptpool", bufs=3))
        rtmp = ctx.enter_context(tc.tile_pool(name="rtmp", bufs=2))
        spool = ctx.enter_context(tc.tile_pool(name="spool", bufs=2))
        opool = ctx.enter_context(tc.tile_pool(name="opool", bufs=6))
        psum_p = ctx.enter_context(tc.tile_pool(name="psum_p", bufs=2, space="PSUM"))
        psum_mix = ctx.enter_context(tc.tile_pool(name="psum_mix", bufs=2, space="PSUM"))
        psum_ot = ctx.enter_context(tc.tile_pool(name="psum_ot", bufs=2, space="PSUM"))

        def ps_tile(pool=None):
            return (pool or psum_p).tile([128, TCH], f32, tag="ps", name="ps")

        def mix_tile():
            return psum_mix.tile([128, H_LOC, TCH], f32, tag="mix", name="mix")

        # --- resident tensors ---
        w_q = persist.tile([128, KO, HD_LOC], f8, tag="w_q")
        w_k = persist.tile([128, KO, HD_LOC], f8, tag="w_k")
        w_vh = persist.tile([128, KO, HD_LOC], f8, tag="w_vh")
        w_vl = persist.tile([128, KO, HD_LOC], f8, tag="w_vl")
        w_oh = persist.tile([128, H_LOC, D], f8, tag="w_oh")
        w_ol = persist.tile([128, H_LOC, D], f8, tag="w_ol")
        kt8 = persist.tile([128, H_LOC, T], f8, tag="kt8")
        vt = persist.tile([128, KO, HD_LOC], bf16, tag="vt")
        ctab = persist.tile([128, T], bf16, tag="ctab")
        stab = persist.tile([128, T], bf16, tag="stab")
        ones = persist.tile([128, 128], bf16, tag="ones")
        tri = persist.tile([128, 128], bf16, tag="tri")

        def issue_x(c):
            """Queue the x piece DMAs for chunk c (weights too on chunk 0).

            DMA order on chunk 0 is latency-critical (the serial fill gates
            the first projections): wq + hi pieces + rope tables first, then
            wk, then the lo pieces / v weights (only needed at v-proj)."""
            cs = c * TCH
            hi, lo = [], []
            if c == 0:
                nc.sync.dma_start(w_q[:], wq_r)
            for kp in range(KO // XP):
                ksl = slice(kp * XP, (kp + 1) * XP)
                xh = xpool.tile([128, XP, TCH], f8, tag="xh", name="xh")
                nc.sync.dma_start(xh[:], x8h_r[:, ksl, cs:cs + TCH])
                hi.append(xh)
            if c == 0:
                nc.sync.dma_start(ctab[:], ct_d[:])
                nc.sync.dma_start(stab[:], st_d[:])
                nc.sync.dma_start(tri[:], tri_d[:])
                nc.sync.dma_start(ones[:], ones_d[:])
                nc.sync.dma_start(w_k[:], wk_r)
            for kp in range(KO // XP):
                ksl = slice(kp * XP, (kp + 1) * XP)
                xl = xpool.tile([128, XP, TCH], f8, tag="xl", name="xl")
                nc.sync.dma_start(xl[:], x8l_r[:, ksl, cs:cs + TCH])
                lo.append(xl)
            if c == 0:
                nc.sync.dma_start(w_vh[:], wvh_r)
                nc.sync.dma_start(w_vl[:], wvl_r)
            return (hi, lo)

        def piece_pair(pieces, xi, kp, tsl=slice(None)):
            """[128, 2, *] DoubleRow operand view for ko-pair kp from XP-wide tiles."""
            o = (kp * 2) % XP
            return pieces[xi][kp * 2 // XP][:, o:o + 2, tsl]

        def proj_chunk(c, pieces, only=None, qc=None):
            """q/k/v projections + RoPE for t-chunk c.

            only="q": just the q projection + its rope (enables starting the
            chunk's early attention j-tiles before k/v exist).
            only="kv": the rest. None: everything."""
            cs = c * TCH
            if only != "kv":
                qc = qpool.tile([128, H_LOC, TCH], f8, tag="qc", name="qc")
            wd = {"q": ((w_q, qc),), "kv": ((w_k, kt8),)}.get(only,
                                                             ((w_q, qc), (w_k, kt8)))
            for w_sb, dst in wd:
                # pre-rope staging tile (bf16) for this src
                pre = rtmp.tile([128, H_LOC, TCH], bf16, tag="pre", name="pre")
                for h in range(H_LOC):
                    # k-groups accumulate in the attention ot pool (idle during
                    # projections); in split mode that pool is live -- psum_p
                    ps = ps_tile(psum_ot if (dst is kt8 and only is None) else None)
                    for kp in range(KP):
                        nc.tensor.matmul(
                            ps,
                            lhsT=w_sb[:, kp * 2:(kp + 1) * 2,
                                      h * 128:(h + 1) * 128],
                            rhs=piece_pair(pieces, 0, kp),
                            start=(kp == 0),
                            stop=(kp == KP - 1),
                            perf_mode=DR,
                        )
                    nc.scalar.activation(out=pre[:, h, :], in_=ps, func=COPY,
                                         scale=1.0 / WSC)
                # RoPE: y = pre*C + roll64(pre)*S', roll via 2 SBUF->SBUF DMAs
                rolled = rtmp.tile([128, H_LOC, TCH], bf16, tag="rolled",
                                   name="rolled")
                for h in range(H_LOC):
                    nc.sync.dma_start(rolled[0:64, h, :], pre[64:128, h, :])
                    nc.sync.dma_start(rolled[64:128, h, :], pre[0:64, h, :])
                a = rtmp.tile([128, H_LOC, TCH], bf16, tag="ra", name="ra")
                b = rtmp.tile([128, H_LOC, TCH], bf16, tag="rb", name="rb")
                for h in range(H_LOC):
                    nc.vector.tensor_mul(out=a[:, h, :], in0=pre[:, h, :],
                                         in1=ctab[:, cs:cs + TCH])
                    nc.vector.tensor_mul(out=b[:, h, :], in0=rolled[:, h, :],
                                         in1=stab[:, cs:cs + TCH])
                dsl = qc[:, :, :] if dst is qc else kt8[:, :, cs:cs + TCH]
                nc.vector.tensor_add(out=dsl, in0=a[:], in1=b[:])

            if only == "q":
                return qc
            # v projection: 3-term compensated fp8 (hi@hi + hi@lo + lo@hi)
            vmix = mix_tile() if only is None else None
            for tt in range(TCH // 128):
                gt = c * (TCH // 128) + tt
                tsl = slice(tt * 128, (tt + 1) * 128)
                if vmix is not None:
                    ps = vmix[:, tt // 2,
                              (tt % 2) * HD_LOC:(tt % 2 + 1) * HD_LOC]
                else:
                    ps = ps_tile()[:, :HD_LOC]
                n = 3 * KP
                i = 0
                for xi, wv in ((0, w_vh), (0, w_vl), (1, w_vh)):
                    for kp in range(KP):
                        nc.tensor.matmul(
                            ps,
                            lhsT=piece_pair(pieces, xi, kp, tsl),
                            rhs=wv[:, kp * 2:(kp + 1) * 2, :],
                            start=(i == 0),
                            stop=(i == n - 1),
                            perf_mode=DR,
                        )
                        i += 1
                nc.vector.tensor_scalar_mul(vt[:, gt, :], ps, 1.0 / WSC)

            return qc

        def attn_span(q0, W, qc, off, yc, jt_lo=0, jt_hi=None,
                      state=None):
            """Causal attention for queries [q0, q0+W), heads interleaved.

            q0 must be 128-aligned; W in {256, 512}. qc holds the chunk's
            roped queries (fp8); off is q0's offset within qc/yc."""
            d0 = q0 // 128          # first diagonal j-tile
            n_jt = d0 + W // 128
            if state is None:
                ots = [ps_tile(psum_ot) for _ in range(H_LOC)]
                vecsums = [spool.tile([128, H_LOC, TCH], bf16,
                                      tag=f"vecsum{par}", name="vecsum")
                           for par in range(2)]
                prev = None
            else:
                ots, vecsums, prev = state
            if jt_hi is None:
                jt_hi = n_jt

            def pv(p):
                jt, pt, lo = p
                for h in range(H_LOC):
                    nc.tensor.matmul(
                        ots[h][:, lo:W],
                        lhsT=vt[:, jt, h * 128:(h + 1) * 128],
                        rhs=pt[:, h, lo:W],
                        start=(jt == 0),
                        stop=(jt == n_jt - 1),
                        skip_group_check=(lo > 0),
                    )

            for jt in range(jt_lo, jt_hi):
                pair = mix_tile()
                m = jt - d0
                # diagonal block: cols < 128m fully masked -- never written,
                # never read (partial-width ops; fp8/bf16 have no narrow-
                # matmul penalty, so trim at full 128 granularity)
                lo = 128 * m if m > 0 else 0
                for h in range(H_LOC):
                    nc.tensor.matmul(
                        pair[:, h, lo:W],
                        lhsT=kt8[:, h, jt * 128:(jt + 1) * 128],
                        rhs=qc[:, h, off + lo:off + W],
                        start=True,
                        stop=True,
                    )
                pt = ptpool.tile([128, H_LOC, TCH], bf16, tag="pt", name="pt")
                # both heads in ONE activation call (strided AP when lo > 0)
                nc.scalar.activation(out=pt[:, :, lo:W], in_=pair[:, :, lo:W],
                                     func=EXP, scale=SCALE)
                for h in range(H_LOC):
                    if m >= 0:
                        nc.vector.tensor_mul(
                            out=pt[:, h, 128 * m:128 * (m + 1)],
                            in0=pt[:, h, 128 * m:128 * (m + 1)],
                            in1=tri[:],
                        )
                # spans starting at q0=0: jt==1 is diagonal (cols < 128
                # unwritten), so a full-width init copy would ingest
                # garbage -- single DVE accumulator there. Other spans
                # split across DVE (even jt) and GPSIMD (odd jt).
                par = jt % 2 if d0 >= 2 else 0
                vs = vecsums[par]
                eng = nc.vector if par == 0 else nc.gpsimd
                if jt < (2 if d0 >= 2 else 1):
                    eng.tensor_copy(out=vs[:, :, :W], in_=pt[:, :, :W])
                else:
                    eng.tensor_add(out=vs[:, :, lo:W], in0=vs[:, :, lo:W],
                                   in1=pt[:, :, lo:W])
                # software pipeline: PV for the PREVIOUS j-tile, so the PE
                # never waits on the exp/mask it just issued
                if prev is not None:
                    pv(prev)
                prev = (jt, pt, lo)
            if jt_hi < n_jt:
                return (ots, vecsums, prev)
            pv(prev)
            # denominator: all-(1/16) matmul -> column sums/16 on all
            # partitions; the 16 resurfaces via the reciprocal so yc = 16*y
            den = mix_tile()
            for h in range(H_LOC):
                if d0 >= 2:
                    nc.tensor.matmul(den[:, h, :W], lhsT=ones,
                                     rhs=vecsums[0][:, h, :W],
                                     start=True, stop=False)
                    nc.tensor.matmul(den[:, h, :W], lhsT=ones,
                                     rhs=vecsums[1][:, h, :W],
                                     start=False, stop=True)
                else:
                    nc.tensor.matmul(den[:, h, :W], lhsT=ones,
                                     rhs=vecsums[0][:, h, :W],
                                     start=True, stop=True)
            recipb = rtmp.tile([128, H_LOC, TCH], f32, tag="recipb",
                               name="recipb")
            nc.vector.reciprocal(out=recipb[:, :, :W], in_=den[:, :, :W])
            for h in range(H_LOC):
                nc.vector.tensor_mul(out=yc[:, h, off:off + W],
                                     in0=ots[h][:, :W],
                                     in1=recipb[:, h, :W])

        def cproj_span(q0, W, yc):
            """Partial c_proj (this core's hd columns) for rows [q0, q0+W).

            yc holds 16*y in f32; split into fp8 hi+lo on GPSIMD, then
            3-term DoubleRow matmuls (both heads packed per instruction)."""
            if q0 == 0:
                nc.sync.dma_start(w_oh[:], woh_r)
                nc.sync.dma_start(w_ol[:], wol_r)
            y8h = ypool.tile([128, H_LOC, TCH], f8, tag="y8h", name="y8h")
            y8l = ypool.tile([128, H_LOC, TCH], f8, tag="y8l", name="y8l")
            nc.gpsimd.tensor_copy(out=y8h[:], in_=yc[:])
            nc.gpsimd.tensor_sub(out=y8l[:], in0=yc[:], in1=y8h[:])
            for tt in range(W // 128):
                gt = q0 // 128 + tt
                tsl = slice(tt * 128, (tt + 1) * 128)
                for nck in range(D // 1024):
                    ps = mix_tile()
                    for half in range(2):
                        dsl = slice(nck * 1024 + half * 512,
                                    nck * 1024 + (half + 1) * 512)
                        for yy, ww, st, sp in ((y8h, w_oh, True, False),
                                               (y8h, w_ol, False, False),
                                               (y8l, w_oh, False, True)):
                            nc.tensor.matmul(
                                ps[:, half, :],
                                lhsT=yy[:, :, tsl],
                                rhs=ww[:, :, dsl],
                                start=st,
                                stop=sp,
                                perf_mode=DR,
                            )
                    ob = opool.tile([128, 2, 512], bf16, tag="ob", name="ob")
                    nc.scalar.activation(out=ob[:], in_=ps[:], func=COPY,
                                         scale=1.0 / (WSC * YSC))
                    nc.sync.dma_start(
                        out_d[gt * 128:(gt + 1) * 128,
                              nck * 1024:(nck + 1) * 1024],
                        ob[:].rearrange("p a b -> p (a b)"),
                    )

        # Emission order: proj(c+1) is emitted between attn(c) and cproj(c)
        # so the next chunk's projection matmuls fill the PE while attn(c)'s
        # denominator/reciprocal/y-split tail runs on DVE/GPSIMD, and
        # cproj(c)'s y8 operands are ready by the time the PE reaches it.
        pieces = issue_x(0)
        qc = proj_chunk(0, pieces)
        nxt = issue_x(1)
        ycs = {}
        for c in range(N_CH - 1):
            yc = ypool.tile([128, H_LOC, TCH], f32, tag="yc", name="yc")
            ycs[c] = yc
            attn_span(c * TCH, TCH, qc, 0, yc)
            pieces = nxt
            if c < N_CH - 2:
                qc = proj_chunk(c + 1, pieces)
                nxt = issue_x(c + 2)
                cproj_span(c * TCH, TCH, ycs[c])
            else:
                # last chunk: q projection + rope first, then its
                # non-diagonal attention (kt/vt chunks 0..2) overlaps the
                # k/v projections; cproj(2) fills the PE in between
                qc = proj_chunk(N_CH - 1, pieces, only="q")
                cproj_span(c * TCH, TCH, ycs[c])
        c = N_CH - 1
        yc = ypool.tile([128, H_LOC, TCH], f32, tag="yc", name="yc")
        st = attn_span(c * TCH, TCH, qc, 0, yc, jt_hi=4 * c)
        proj_chunk(c, pieces, only="kv", qc=qc)
        attn_span(c * TCH, TCH, qc, 0, yc, jt_lo=4 * c, state=st)
        cproj_span(c * TCH, TCH, yc)

    nc.compile()
    _CACHE["nc"] = nc
    return nc


def host_inputs(x, Wq, Wk, Wv, Wo):
    """Per-core input dicts (host-side shard + transpose + fp8 split)."""
    import ml_dtypes

    F8 = ml_dtypes.float8_e4m3
    BF = ml_dtypes.bfloat16

    def f8_of(a):
        return np.asarray(a, np.float32).astype(F8)

    def f8_split(a):
        hi = f8_of(a)
        lo = (np.asarray(a, np.float32) - hi.astype(np.float32)).astype(F8)
        return hi, lo

    x2 = np.ascontiguousarray(x.reshape(T, D).T).astype(np.float32)  # (D, T)
    x8h, x8l = f8_split(x2)

    af = (1.0 / 1024.0) ** np.linspace(0.0, 1.0, DH // 4, dtype=np.float32)
    af = np.concatenate([af, np.zeros(DH // 4, np.float32)])         # (64,)
    theta = np.arange(T, dtype=np.float32)[:, None] * af[None, :]    # (T, 64)
    cos = np.cos(theta).T.astype(np.float32)                         # (64, T)
    sin = np.sin(theta).T.astype(np.float32)
    ctab = np.concatenate([cos, cos], axis=0).astype(BF)             # (128, T)
    stab = np.concatenate([sin, -sin], axis=0).astype(BF)

    ones = np.full((128, 128), 1.0 / YSC, BF)
    tri = np.triu(np.ones((128, 128), np.float32)).astype(BF)  # tri[j,i]=i>=j

    def wlay(a):
        # (KO*128, m) -> partition-major (128, KO*m) so one DMA loads it
        m = a.shape[1]
        return np.ascontiguousarray(
            a.reshape(KO, 128, m).transpose(1, 0, 2).reshape(128, KO * m))

    def olay(a):
        # (H_LOC*128, D) -> (128, H_LOC*D)
        return np.ascontiguousarray(
            a.reshape(H_LOC, 128, D).transpose(1, 0, 2).reshape(128, H_LOC * D))

    shared = {
        "x8h": x8h, "x8l": x8l, "ctab": ctab, "stab": stab,
        "ones": ones, "tri": tri,
    }
    in_maps = []
    for c in range(N_CORES):
        sl = slice(c * HD_LOC, (c + 1) * HD_LOC)
        wv8h, wv8l = f8_split(Wv[sl, :].T * WSC)
        wo8h, wo8l = f8_split((Wo[:, sl] / 3.0).T * WSC)
        in_maps.append({
            **shared,
            "wq8": wlay(f8_of(Wq[sl, :].T * WSC)),
            "wk8": wlay(f8_of(Wk[sl, :].T * WSC)),
            "wv8h": wlay(wv8h), "wv8l": wlay(wv8l),
            "wo8h": olay(wo8h), "wo8l": olay(wo8l),
        })
    return in_maps


def _get_runner():
    """Build the program + a persistent jitted SPMD executable (once)."""
    if "runner" in _CACHE:
        return _CACHE["runner"]

    import jax
    import concourse.mybir as mybir
    from concourse.bass2jax import (
        _bass_exec_p,
        install_neuronx_cc_hook,
        partition_id_tensor,
    )
    from jax.experimental.shard_map import shard_map
    from jax.sharding import Mesh, PartitionSpec

    nc = build_program()
    install_neuronx_cc_hook()
    assert nc.dbg_addr is None
    pid_name = nc.partition_id_tensor.name if nc.partition_id_tensor else None

    in_names, out_names, out_avals, zero_outs = [], [], [], []
    for alloc in nc.m.functions[0].allocations:
        if not isinstance(alloc, mybir.MemoryLocationSet):
            continue
        name = alloc.memorylocations[0].name
        if alloc.kind == "ExternalInput":
            if name != pid_name:
                in_names.append(name)
        elif alloc.kind == "ExternalOutput":
            out_names.append(name)
            shape = tuple(alloc.tensor_shape)
            dtype = mybir.dt.np(alloc.dtype)
            out_avals.append(jax.core.ShapedArray(shape, dtype))
            zero_outs.append(np.zeros(shape, dtype))
    n_params = len(in_names)
    all_names = list(in_names) + list(out_names)
    if pid_name is not None:
        all_names.append(pid_name)
    donate = tuple(range(n_params, n_params + len(out_names)))

    def _body(*args):
        operands = list(args)
        if pid_name is not None:
            operands.append(partition_id_tensor())
        outs = _bass_exec_p.bind(
            *operands,
            out_avals=tuple(out_avals),
            in_names=tuple(all_names),
            out_names=tuple(out_names),
            lowering_input_output_aliases=(),
            sim_require_finite=True,
            sim_require_nnan=True,
            nc=nc,
        )
        return tuple(outs)

    devices = jax.devices()[:N_CORES]
    mesh = Mesh(np.asarray(devices), ("core",))
    in_specs = (PartitionSpec("core"),) * (n_params + len(out_names))
    out_specs = (PartitionSpec("core"),) * len(out_names)
    fn = jax.jit(
        shard_map(_body, mesh=mesh, in_specs=in_specs, out_specs=out_specs,
                  check_rep=False),
        donate_argnums=donate,
        keep_unused=True,
    )
    runner = (fn, in_names, out_names, out_avals, zero_outs)
    _CACHE["runner"] = runner
    return runner


def run_spmd(in_maps):
    """Execute the SPMD program; returns per-core output dicts."""
    fn, in_names, out_names, out_avals, zero_outs = _get_runner()
    concat_in = [
        np.concatenate([np.asarray(in_maps[c][n]) for c in range(N_CORES)], axis=0)
        for n in in_names
    ]
    concat_zeros = [
        np.zeros((N_CORES * z.shape[0], *z.shape[1:]), z.dtype) for z in zero_outs
    ]
    out_arrs = fn(*concat_in, *concat_zeros)
    return [
        {n: np.asarray(out_arrs[i]).reshape(N_CORES, *out_avals[i].shape)[c]
         for i, n in enumerate(out_names)}
        for c in range(N_CORES)
    ]


def kernel(x, Wq, Wk, Wv, Wo):
    in_maps = host_inputs(np.asarray(x), np.asarray(Wq), np.asarray(Wk),
                          np.asarray(Wv), np.asarray(Wo))
    results = run_spmd(in_maps)
    out = results[0]["outp"].astype(np.float64)
    for c in range(1, N_CORES):
        out += results[c]["outp"].astype(np.float64)
    return out.astype(np.float32).reshape(1, T, D)
